# revision 14
# baseline (speedup 1.0000x reference)
"""Trainium2 Bass kernel for the MANTIS quantum-circuit-loss nn.Module.

Shapes (hardcoded): B=128, L=16, M=32, P=4.  8 NeuronCores, batch-sharded
(16 batch elements per core).

Math
----
Let j = (m, p) flattened (M*P = 128 == partition count) and
    A[b, l, j] = theta[l, j] + scal[p(j)] * input_ds[b, l]
    CA = cos(A), SA = sin(A)

prob term:      amp[b]  = sum_j coef_j prod_l CA[b,l,j]
normalization:  norm[b] = sum_{j,k} coef_j coef_k prod_l cos(A[b,l,j]-A[b,l,k])

Using cos(a-b) = cos a cos b + sin a sin b, norm[b] is the squared norm of a
sum of 128 product states in the 2^16-dim site space.  Split the 16 sites
into two groups of 8; for each group build the 256 branch-product vectors
    U_g[j, T] = prod_{l in g} X_{T_l}[b, l, j],  X_0 = CA, X_1 = SA
by log-doubling (elementwise multiplies, bf16).  With coef folded into
U1 (site 0):
    D_b[T1, T2] = sum_j (c U1)[j, T1] U2[j, T2]    (PE matmul, K = 128)
    norm[b] = sum_{T1,T2} D_b^2
    amp[b]  = sum_j (c U1)[j, 0] U2[j, 0]

cos AND sin come from ONE Sin activation over a double-width tile
    arg = [pi/2 - A | A],  cs = Sin(arg) = [cos A | sin A]
so the only ACT table ever needed is trig_and_small (contains Sin AND
Square) -> exactly one ACT_TABLE_LOAD in the program.

Per-core output is [1, 34]:
    out[0, 0:16]  = norm[b]  (sum of squares of D_b)
    out[0, 16:32] = amp[b]
    out[0, 32]    = reg_total (REG_C*var(coef) + theta var terms)
The host computes loss = mean_b -(ln(amp^2 + EPS*norm) - ln(norm)) + reg
in float64 (exact; cheaper and more accurate than on-device Ln).
"""

import math
import os

import numpy as np

import concourse.bacc as bacc
import concourse.bass as bass
import concourse.mybir as mybir
import concourse.tile as tile

B, L, M, P = 128, 16, 32, 4
NCORES = 8
BLOC = B // NCORES  # 16 batch elements per core
J = M * P  # 128
EPS = 1e-20
REG_C = 0.01
REG_THETA_M = 0.01
REG_THETA_P = 0.01

F32 = mybir.dt.float32
BF16 = mybir.dt.bfloat16
AF = mybir.ActivationFunctionType
ALU = mybir.AluOpType

# dtype of the doubling chain (l1/l2/U) and the D matmuls.  bf16 validated:
# final-loss rel err 4.8e-5 vs the 2e-2 gate (sum-of-65536-squares washes
# out the rounding; amp has no catastrophic cancellation on these inputs).
MM_DT = BF16 if os.environ.get("MANTIS_MM_DT", "bf16") == "bf16" else F32
CHUNKS = [int(x) for x in os.environ.get("MANTIS_CHUNKS", "1,1,2,4,4,4").split(",")]
# which batch elements' square+reduce runs on DVE (rest on ACT)
DVE_SQ_IDS = [int(x) for x in os.environ.get("MANTIS_DVE_SQ", "13,15").split(",") if x]
# which (chunk, group) U-builds run on Pool, as "c.g" pairs
POOL_U = set(os.environ.get("MANTIS_POOL_U", "3.1,4.1,5.1").split(",")) - {""}

# params column layout (head of the input blob)
PC_THETA = 0  # 16 cols: theta_t[j, l]
PC_PHT = 16  # 16 cols: pi/2 - theta_t[j, l]
PC_COEF = 32  # 1 col
PC_SCAL = 33  # 1 col: pi / 2^(p(j)+1)
PC_NSCAL = 34  # 1 col: -scal
PC_DVEC = 35  # 1 col: 1/n for the var terms (rows 0:37)
PC_MASK = 36  # 37 cols: [ones | mask_p(4) | mask_m(32)]
PC_REGW = 73  # 17 cols: reg weights (rows 0:37)
PC_INDS = 90  # 256 cols: input_ds slice, broadcast to all partitions
NCOLS = PC_INDS + BLOC * L  # 346

# FIN column layout: [128, 32]; 0:16 per-b sumsq partials, 16:32 amp partials
F_COLS = 32


def build_params() -> np.ndarray:
    pr = np.zeros((J, NCOLS), dtype=np.float32)
    sf = (np.pi / 2.0 ** (np.arange(P) + 1.0)).astype(np.float32)
    pr[:, PC_SCAL] = np.tile(sf, M)
    pr[:, PC_NSCAL] = -pr[:, PC_SCAL]
    # dvec: 1/n divisors for var terms
    pr[0, PC_DVEC] = 1.0 / 128.0
    pr[1:5, PC_DVEC] = 1.0 / 32.0
    pr[5:37, PC_DVEC] = 1.0 / 4.0
    # masks
    pr[:, PC_MASK] = 1.0  # ones
    jj = np.arange(J)
    pr[jj, PC_MASK + 1 + (jj % 4)] = 1.0  # mask_p
    pr[jj, PC_MASK + 5 + (jj // 4)] = 1.0  # mask_m
    # REGW (rows 0:37): weight for each cell of (S^2/n - SS) so that
    # sum(REGW * (S^2/n - SS)) == reg_total.  var = (SS - S^2/n)/(n-1), so
    # weight = -reg_coeff * mean_factor / (n-1).
    pr[0, PC_REGW + 16] = -REG_C / 127.0
    pr[1:5, PC_REGW : PC_REGW + 16] = -REG_THETA_M / 64.0 / 31.0
    pr[5:37, PC_REGW : PC_REGW + 16] = -REG_THETA_P / 512.0 / 3.0
    return pr


def build_program():
    """Build the SPMD Bass/Tile program (identical on all 8 cores)."""
    nc = bacc.Bacc(
        "TRN2",
        target_bir_lowering=False,
        debug=False,
        num_devices=NCORES,
    )
    blob_d = nc.dram_tensor("blob", [J, NCOLS], F32, kind="ExternalInput")
    out_d = nc.dram_tensor("out", [1, 49], F32, kind="ExternalOutput")

    with tile.TileContext(nc) as tc:
        with (
            tc.tile_pool(name="const", bufs=1) as cpool,
            tc.tile_pool(name="work", bufs=1) as wpool,
            tc.tile_pool(name="dps", bufs=5, space=bass.MemorySpace.PSUM) as dpool,
            tc.tile_pool(name="fps", bufs=1, space=bass.MemorySpace.PSUM) as fpool,
        ):
            _emit(nc, tc, cpool, wpool, dpool, fpool, blob_d, out_d)
    nc.compile()
    return nc


def _emit(nc, tc, cpool, wpool, dpool, fpool, blob_d, out_d):
    params = cpool.tile([J, NCOLS], F32, tag="params")
    nc.sync.dma_start(params[:], blob_d[:, :])

    theta_ap = params[:, PC_THETA : PC_THETA + L]
    pht_ap = params[:, PC_PHT : PC_PHT + L]
    coef_ap = params[:, PC_COEF : PC_COEF + 1]
    scal_ap = params[:, PC_SCAL : PC_SCAL + 1]
    nscal_ap = params[:, PC_NSCAL : PC_NSCAL + 1]
    inds = params[:, PC_INDS : PC_INDS + BLOC * L]
    in_bc = inds.rearrange("j (i l) -> j i l", i=BLOC, l=L)

    # --- ARG [J, 512]: cols 0:256 = pi/2 - A (cos arg), 256:512 = A (sin arg)
    # A = theta + scal*inds; both halves stay within the Sin table's range:
    # A in (-1, 2.58)  =>  pi/2 - A in (-1.01, 2.58).
    arg = wpool.tile([J, 2 * BLOC * L], F32, tag="arg")
    argv = arg[:].rearrange("j (t i l) -> j t i l", t=2, i=BLOC, l=L)
    th_bc = theta_ap.unsqueeze(1).broadcast_to([J, BLOC, L])
    pht_bc = pht_ap.unsqueeze(1).broadcast_to([J, BLOC, L])
    nc.vector.scalar_tensor_tensor(
        out=argv[:, 0], in0=in_bc, scalar=nscal_ap, in1=pht_bc,
        op0=ALU.mult, op1=ALU.add,
    )
    nc.vector.scalar_tensor_tensor(
        out=argv[:, 1], in0=in_bc, scalar=scal_ap, in1=th_bc,
        op0=ALU.mult, op1=ALU.add,
    )

    # --- CS[j, (t,i,l)]: t=0 -> cos(A), t=1 -> sin(A).  Two Sin activations
    # (one per half so the first can start as soon as arg half 0 is ready);
    # the only ACT table the whole program needs is trig_and_small.
    cs = wpool.tile([J, 2 * BLOC * L], F32, tag="cs")
    nc.scalar.activation(cs[:, 0 : BLOC * L], arg[:, 0 : BLOC * L], AF.Sin)
    nc.scalar.activation(cs[:, BLOC * L :], arg[:, BLOC * L :], AF.Sin)
    cs_v = cs[:].rearrange("j (t i l) -> j t i l", t=2, i=BLOC, l=L)

    # fold coef into site l=0 (both branches) => every T1 combo of group 0
    # carries exactly one coef_j factor.
    nc.vector.tensor_scalar_mul(cs_v[:, :, :, 0:1], cs_v[:, :, :, 0:1], coef_ap)

    final = wpool.tile([1, 49], F32, tag="final")

    # --- reg head: fin_r = [theta(16) | coef(1) | squares(17)], masked matmul.
    # Square on Pool (not ACT): an ACT Square here could be scheduled before
    # the Sin and would force an extra activation-table switch.
    fin_r = wpool.tile([J, 34], F32, tag="fin_r")
    nc.vector.tensor_copy(fin_r[:, 0:17], params[:, 0:17])
    nc.gpsimd.tensor_tensor(
        out=fin_r[:, 17:34], in0=params[:, 0:17], in1=params[:, 0:17], op=ALU.mult
    )
    fout_r = fpool.tile([37, 34], F32, tag="fout_r")
    nc.tensor.matmul(fout_r[:], params[:, PC_MASK : PC_MASK + 37], fin_r[:])

    # --- doubling: L1 (site pairs, 4 combos), L2 (quads, 16 combos), bf16
    l1 = [wpool.tile([J, BLOC * 16], MM_DT, tag=f"l1_{g}", name=f"l1_{g}") for g in range(2)]
    l2 = [wpool.tile([J, BLOC * 32], MM_DT, tag=f"l2_{g}", name=f"l2_{g}") for g in range(2)]
    for g in range(2):
        lo = g * 8  # first site of the group
        o1all = l1[g][:].rearrange(
            "j (i s t1 t2) -> j i s t1 t2", i=BLOC, s=4, t1=2, t2=2
        )
        for t1 in range(2):
            in1 = (
                cs_v[:, t1, :, lo : lo + 8 : 2]
                .unsqueeze(3)
                .broadcast_to([J, BLOC, 4, 2])
            )
            in2 = cs_v[:, :, :, lo + 1 : lo + 8 : 2].transpose([0, 2, 3, 1])
            o1 = o1all[:, :, :, t1, :]
            nc.vector.tensor_tensor(out=o1, in0=in1, in1=in2, op=ALU.mult)
        l1v = l1[g][:].rearrange("j (i s c) -> j i s c", i=BLOC, s=4, c=4)
        o2all = l2[g][:].rearrange(
            "j (i d q1 q2) -> j i d q1 q2", i=BLOC, d=2, q1=4, q2=4
        )
        for d in range(2):
            in1 = l1v[:, :, 2 * d, :].unsqueeze(3).broadcast_to([J, BLOC, 4, 4])
            in2 = l1v[:, :, 2 * d + 1, :].unsqueeze(2).broadcast_to([J, BLOC, 4, 4])
            o2 = o2all[:, :, d, :, :]
            nc.vector.tensor_tensor(out=o2, in0=in1, in1=in2, op=ALU.mult)

    # --- L3 chunked by batch; per-chunk U tiles so PE/ACT pipeline per chunk
    fin = wpool.tile([J, F_COLS], F32, tag="fin")  # 0:16 sumsq, 16:32 amp
    chunk_sizes = CHUNKS
    assert sum(chunk_sizes) == BLOC
    sq_dve = set(DVE_SQ_IDS)
    with tc.tile_pool(name="dsqp", bufs=2) as spool:
        i0 = 0
        for c, csz in enumerate(chunk_sizes):
            cw = csz * 256
            uc = [
                wpool.tile([J, cw], MM_DT, tag=f"u_{g}_{c}", name=f"u_{g}_{c}")
                for g in range(2)
            ]
            for g in range(2):
                l2v = l2[g][:].rearrange(
                    "j (i d c16) -> j i d c16", i=BLOC, d=2, c16=16
                )
                in1 = (
                    l2v[:, i0 : i0 + csz, 0, :]
                    .unsqueeze(3)
                    .broadcast_to([J, csz, 16, 16])
                )
                in2 = (
                    l2v[:, i0 : i0 + csz, 1, :]
                    .unsqueeze(2)
                    .broadcast_to([J, csz, 16, 16])
                )
                ov = uc[g][:].rearrange(
                    "j (i u1 u2) -> j i u1 u2", i=csz, u1=16, u2=16
                )
                eng = nc.gpsimd if f"{c}.{g}" in POOL_U else nc.vector
                eng.tensor_tensor(out=ov, in0=in1, in1=in2, op=ALU.mult)

            # amp partials for this chunk: fin[:, 16+i] = cU1[j,i,0]*U2[j,i,0]
            u1v = uc[0][:].rearrange("j (i t) -> j i t", i=csz, t=256)
            u2v = uc[1][:].rearrange("j (i t) -> j i t", i=csz, t=256)
            nc.vector.tensor_tensor(
                out=fin[:, 16 + i0 : 16 + i0 + csz],
                in0=u1v[:, :, 0], in1=u2v[:, :, 0], op=ALU.mult,
            )

            # D matmuls + square/accum for this chunk's batch elements
            for k in range(csz):
                i = i0 + k
                dt = dpool.tile([J, 512], F32, tag="D")
                rhs = uc[1][:, k * 256 : (k + 1) * 256]
                for h in range(2):
                    lhsT = uc[0][:, k * 256 + h * 128 : k * 256 + (h + 1) * 128]
                    nc.tensor.matmul(dt[:, h * 256 : (h + 1) * 256], lhsT, rhs)
                # norm partials: fin[:, i] = sum_T2 D[T1row, :]^2
                if i in sq_dve:
                    eng = nc.vector
                    # stage through SBUF in bf16 (PSUM dual-read is illegal;
                    # packed bf16 SBUF can hit the DVE 2x path)
                    dsq = spool.tile([J, 512], BF16, tag="dsq", name="dsq")
                    eng.tensor_copy(dsq[:], dt[:])
                    eng.scalar_tensor_tensor(
                        out=dsq[:], in0=dsq[:], scalar=1.0, in1=dsq[:],
                        op0=ALU.mult, op1=ALU.mult,
                        accum_out=fin[:, i : i + 1],
                    )
                else:
                    nc.scalar.activation(
                        dt[:], dt[:], AF.Square,
                        accum_out=fin[:, i : i + 1],
                    )
            i0 += csz

    # --- tail: ones-matmul reduces fin over partitions -> [1, 32]
    fout = fpool.tile([1, F_COLS], F32, tag="fout")
    nc.tensor.matmul(fout[:], params[:, PC_MASK : PC_MASK + 1], fin[:])
    nc.vector.tensor_copy(final[0:1, 0:32], fout[0:1, 0:32])

    # --- reg tail (off critical path; v-chain on Pool; Pool supports only
    # TensorTensor/TensorCopy, so the free-dim reduction goes through a
    # ones-matmul to [1,17] and the host sums those 17 values)
    sv = wpool.tile([37, 34], F32, tag="sv")
    nc.vector.tensor_copy(sv[:], fout_r[0:37, 0:34])
    v1 = wpool.tile([37, 17], F32, tag="v1")
    nc.gpsimd.tensor_tensor(out=v1[:], in0=sv[:, 0:17], in1=sv[:, 0:17], op=ALU.mult)
    v2 = wpool.tile([37, 17], F32, tag="v2")
    dvec_bc = params[0:37, PC_DVEC : PC_DVEC + 1].broadcast_to([37, 17])
    nc.gpsimd.tensor_tensor(out=v2[:], in0=v1[:], in1=dvec_bc, op=ALU.mult)
    v3 = wpool.tile([37, 17], F32, tag="v3")
    nc.gpsimd.tensor_tensor(out=v3[:], in0=v2[:], in1=sv[:, 17:34], op=ALU.subtract)
    v4 = wpool.tile([37, 17], F32, tag="v4")
    nc.gpsimd.tensor_tensor(
        out=v4[:], in0=v3[:],
        in1=params[0:37, PC_REGW : PC_REGW + 17], op=ALU.mult,
    )
    rt = fpool.tile([1, 17], F32, tag="rt")
    nc.tensor.matmul(rt[:], params[0:37, PC_MASK : PC_MASK + 1], v4[:])
    nc.vector.tensor_copy(final[0:1, 32:49], rt[:])

    nc.sync.dma_start(out_d[:, :], final[:])


def make_in_maps(input_ds, theta, coef):
    input_ds = np.asarray(input_ds, dtype=np.float32)
    theta = np.asarray(theta, dtype=np.float32)
    coef = np.asarray(coef, dtype=np.float32)
    pr = build_params()
    th = theta.transpose(1, 2, 0).reshape(J, L)
    pr[:, PC_THETA : PC_THETA + L] = th
    pr[:, PC_PHT : PC_PHT + L] = (np.float32(np.pi / 2.0) - th).astype(np.float32)
    pr[:, PC_COEF] = coef.reshape(J)
    in_maps = []
    for c in range(NCORES):
        blob = pr.copy()
        sl = input_ds[c * BLOC : (c + 1) * BLOC, :].reshape(1, BLOC * L)
        blob[:, PC_INDS:] = sl
        in_maps.append({"blob": blob})
    return in_maps


_NC_CACHE = None


def _get_program():
    global _NC_CACHE
    if _NC_CACHE is None:
        _NC_CACHE = build_program()
    return _NC_CACHE


def combine_outputs(results):
    """Host-side tail in float64: ln / mean over batch + reg term."""
    loss = 0.0
    for c in range(NCORES):
        o = np.asarray(results[c]["out"], dtype=np.float64)
        norm = o[0, 0:16]
        amp = o[0, 16:32]
        prob = amp * amp
        loss += float(np.sum(-(np.log(prob + EPS * norm) - np.log(norm))))
    loss /= float(B)
    loss += float(np.asarray(results[0]["out"], dtype=np.float64)[0, 32:49].sum())
    return np.float32(loss)


def kernel(input_ds, theta, coef):
    from concourse.bass_utils import run_bass_kernel_spmd

    nc = _get_program()
    in_maps = make_in_maps(input_ds, theta, coef)
    res = run_bass_kernel_spmd(nc, in_maps, core_ids=list(range(NCORES)))
    return combine_outputs(res.results)


# revision 15
# speedup vs baseline: 1.0617x; 1.0617x over previous
"""Trainium2 Bass kernel for the MANTIS quantum-circuit-loss nn.Module.

Shapes (hardcoded): B=128, L=16, M=32, P=4.  8 NeuronCores, batch-sharded
(16 batch elements per core).

Math
----
Let j = (m, p) flattened (M*P = 128 == partition count) and
    A[b, l, j] = theta[l, j] + scal[p(j)] * input_ds[b, l]
    CA = cos(A), SA = sin(A)

prob term:      amp[b]  = sum_j coef_j prod_l CA[b,l,j]
normalization:  norm[b] = sum_{j,k} coef_j coef_k prod_l cos(A[b,l,j]-A[b,l,k])

Using cos(a-b) = cos a cos b + sin a sin b, norm[b] is the squared norm of a
sum of 128 product states in the 2^16-dim site space.  Split the 16 sites
into two groups of 8; for each group build the 256 branch-product vectors
    U_g[j, T] = prod_{l in g} X_{T_l}[b, l, j],  X_0 = CA, X_1 = SA
by log-doubling (elementwise multiplies, bf16).  With coef folded into
U1 (site 0):
    D_b[T1, T2] = sum_j (c U1)[j, T1] U2[j, T2]    (PE matmul, K = 128)
    norm[b] = sum_{T1,T2} D_b^2
    amp[b]  = sum_j (c U1)[j, 0] U2[j, 0]

cos AND sin come from ONE Sin activation over a double-width tile
    arg = [pi/2 - A | A],  cs = Sin(arg) = [cos A | sin A]
so the only ACT table ever needed is trig_and_small (contains Sin AND
Square) -> exactly one ACT_TABLE_LOAD in the program.

Per-core output is [1, 34]:
    out[0, 0:16]  = norm[b]  (sum of squares of D_b)
    out[0, 16:32] = amp[b]
    out[0, 32]    = reg_total (REG_C*var(coef) + theta var terms)
The host computes loss = mean_b -(ln(amp^2 + EPS*norm) - ln(norm)) + reg
in float64 (exact; cheaper and more accurate than on-device Ln).
"""

import math
import os

import numpy as np

import concourse.bacc as bacc
import concourse.bass as bass
import concourse.mybir as mybir
import concourse.tile as tile

B, L, M, P = 128, 16, 32, 4
NCORES = 8
BLOC = B // NCORES  # 16 batch elements per core
J = M * P  # 128
EPS = 1e-20
REG_C = 0.01
REG_THETA_M = 0.01
REG_THETA_P = 0.01

F32 = mybir.dt.float32
BF16 = mybir.dt.bfloat16
AF = mybir.ActivationFunctionType
ALU = mybir.AluOpType

# dtype of the doubling chain (l1/l2/U) and the D matmuls.  bf16 validated:
# final-loss rel err 4.8e-5 vs the 2e-2 gate (sum-of-65536-squares washes
# out the rounding; amp has no catastrophic cancellation on these inputs).
MM_DT = BF16 if os.environ.get("MANTIS_MM_DT", "bf16") == "bf16" else F32
CHUNKS = [int(x) for x in os.environ.get("MANTIS_CHUNKS", "1,1,2,4,4,4").split(",")]
# which batch elements' square+reduce runs on DVE (rest on ACT)
DVE_SQ_IDS = [int(x) for x in os.environ.get("MANTIS_DVE_SQ", "13,15").split(",") if x]
# which (chunk, group) U-builds run on Pool, as "c.g" pairs
POOL_U = set(os.environ.get("MANTIS_POOL_U", "").split(",")) - {""}

# params column layout (head of the input blob)
PC_THETA = 0  # 16 cols: theta_t[j, l]
PC_PHT = 16  # 16 cols: pi/2 - theta_t[j, l]
PC_COEF = 32  # 1 col
PC_SCAL = 33  # 1 col: pi / 2^(p(j)+1)
PC_NSCAL = 34  # 1 col: -scal
PC_DVEC = 35  # 1 col: 1/n for the var terms (rows 0:37)
PC_MASK = 36  # 37 cols: [ones | mask_p(4) | mask_m(32)]
PC_REGW = 73  # 17 cols: reg weights (rows 0:37)
PC_INDS = 90  # 256 cols: input_ds slice, broadcast to all partitions
NCOLS = PC_INDS + BLOC * L  # 346

# FIN column layout: [128, 32]; 0:16 per-b sumsq partials, 16:32 amp partials
F_COLS = 32


def build_params() -> np.ndarray:
    pr = np.zeros((J, NCOLS), dtype=np.float32)
    sf = (np.pi / 2.0 ** (np.arange(P) + 1.0)).astype(np.float32)
    pr[:, PC_SCAL] = np.tile(sf, M)
    pr[:, PC_NSCAL] = -pr[:, PC_SCAL]
    # dvec: 1/n divisors for var terms
    pr[0, PC_DVEC] = 1.0 / 128.0
    pr[1:5, PC_DVEC] = 1.0 / 32.0
    pr[5:37, PC_DVEC] = 1.0 / 4.0
    # masks
    pr[:, PC_MASK] = 1.0  # ones
    jj = np.arange(J)
    pr[jj, PC_MASK + 1 + (jj % 4)] = 1.0  # mask_p
    pr[jj, PC_MASK + 5 + (jj // 4)] = 1.0  # mask_m
    # REGW (rows 0:37): weight for each cell of (S^2/n - SS) so that
    # sum(REGW * (S^2/n - SS)) == reg_total.  var = (SS - S^2/n)/(n-1), so
    # weight = -reg_coeff * mean_factor / (n-1).
    pr[0, PC_REGW + 16] = -REG_C / 127.0
    pr[1:5, PC_REGW : PC_REGW + 16] = -REG_THETA_M / 64.0 / 31.0
    pr[5:37, PC_REGW : PC_REGW + 16] = -REG_THETA_P / 512.0 / 3.0
    return pr


def build_program():
    """Build the SPMD Bass/Tile program (identical on all 8 cores)."""
    nc = bacc.Bacc(
        "TRN2",
        target_bir_lowering=False,
        debug=False,
        num_devices=NCORES,
    )
    blob_d = nc.dram_tensor("blob", [J, NCOLS], F32, kind="ExternalInput")
    out_d = nc.dram_tensor("out", [1, 49], F32, kind="ExternalOutput")

    with tile.TileContext(nc) as tc:
        with (
            tc.tile_pool(name="const", bufs=1) as cpool,
            tc.tile_pool(name="work", bufs=1) as wpool,
            tc.tile_pool(name="dps", bufs=5, space=bass.MemorySpace.PSUM) as dpool,
            tc.tile_pool(name="fps", bufs=1, space=bass.MemorySpace.PSUM) as fpool,
        ):
            _emit(nc, tc, cpool, wpool, dpool, fpool, blob_d, out_d)
    nc.compile()
    return nc


def _emit(nc, tc, cpool, wpool, dpool, fpool, blob_d, out_d):
    params = cpool.tile([J, NCOLS], F32, tag="params")
    nc.sync.dma_start(params[:], blob_d[:, :])

    theta_ap = params[:, PC_THETA : PC_THETA + L]
    pht_ap = params[:, PC_PHT : PC_PHT + L]
    coef_ap = params[:, PC_COEF : PC_COEF + 1]
    scal_ap = params[:, PC_SCAL : PC_SCAL + 1]
    nscal_ap = params[:, PC_NSCAL : PC_NSCAL + 1]
    inds = params[:, PC_INDS : PC_INDS + BLOC * L]
    in_bc = inds.rearrange("j (i l) -> j i l", i=BLOC, l=L)

    # --- ARG [J, 512]: cols 0:256 = pi/2 - A (cos arg), 256:512 = A (sin arg)
    # A = theta + scal*inds; both halves stay within the Sin table's range:
    # A in (-1, 2.58)  =>  pi/2 - A in (-1.01, 2.58).
    arg = wpool.tile([J, 2 * BLOC * L], F32, tag="arg")
    argv = arg[:].rearrange("j (t i l) -> j t i l", t=2, i=BLOC, l=L)
    th_bc = theta_ap.unsqueeze(1).broadcast_to([J, BLOC, L])
    pht_bc = pht_ap.unsqueeze(1).broadcast_to([J, BLOC, L])
    nc.vector.scalar_tensor_tensor(
        out=argv[:, 0], in0=in_bc, scalar=nscal_ap, in1=pht_bc,
        op0=ALU.mult, op1=ALU.add,
    )
    nc.vector.scalar_tensor_tensor(
        out=argv[:, 1], in0=in_bc, scalar=scal_ap, in1=th_bc,
        op0=ALU.mult, op1=ALU.add,
    )

    # --- CS[j, (t,i,l)]: t=0 -> cos(A), t=1 -> sin(A).  Two Sin activations
    # (one per half so the first can start as soon as arg half 0 is ready);
    # the only ACT table the whole program needs is trig_and_small.
    cs = wpool.tile([J, 2 * BLOC * L], F32, tag="cs")
    nc.scalar.activation(cs[:, 0 : BLOC * L], arg[:, 0 : BLOC * L], AF.Sin)
    nc.scalar.activation(cs[:, BLOC * L :], arg[:, BLOC * L :], AF.Sin)
    cs_v = cs[:].rearrange("j (t i l) -> j t i l", t=2, i=BLOC, l=L)

    # fold coef into site l=0 (both branches) => every T1 combo of group 0
    # carries exactly one coef_j factor.
    nc.vector.tensor_scalar_mul(cs_v[:, :, :, 0:1], cs_v[:, :, :, 0:1], coef_ap)

    final = wpool.tile([1, 49], F32, tag="final")

    # --- reg head: fin_r = [theta(16) | coef(1) | squares(17)], masked matmul.
    # Square on Pool (not ACT): an ACT Square here could be scheduled before
    # the Sin and would force an extra activation-table switch.
    fin_r = wpool.tile([J, 34], F32, tag="fin_r")
    nc.vector.tensor_copy(fin_r[:, 0:17], params[:, 0:17])
    nc.gpsimd.tensor_tensor(
        out=fin_r[:, 17:34], in0=params[:, 0:17], in1=params[:, 0:17], op=ALU.mult
    )
    fout_r = fpool.tile([37, 34], F32, tag="fout_r")
    nc.tensor.matmul(fout_r[:], params[:, PC_MASK : PC_MASK + 37], fin_r[:])

    # --- doubling: L1 (site pairs, 4 combos), L2 (quads, 16 combos), bf16
    l1 = [wpool.tile([J, BLOC * 16], MM_DT, tag=f"l1_{g}", name=f"l1_{g}") for g in range(2)]
    l2 = [wpool.tile([J, BLOC * 32], MM_DT, tag=f"l2_{g}", name=f"l2_{g}") for g in range(2)]
    for g in range(2):
        lo = g * 8  # first site of the group
        o1all = l1[g][:].rearrange(
            "j (i s t1 t2) -> j i s t1 t2", i=BLOC, s=4, t1=2, t2=2
        )
        for t1 in range(2):
            in1 = (
                cs_v[:, t1, :, lo : lo + 8 : 2]
                .unsqueeze(3)
                .broadcast_to([J, BLOC, 4, 2])
            )
            in2 = cs_v[:, :, :, lo + 1 : lo + 8 : 2].transpose([0, 2, 3, 1])
            o1 = o1all[:, :, :, t1, :]
            nc.vector.tensor_tensor(out=o1, in0=in1, in1=in2, op=ALU.mult)
        l1v = l1[g][:].rearrange("j (i s c) -> j i s c", i=BLOC, s=4, c=4)
        o2all = l2[g][:].rearrange(
            "j (i d q1 q2) -> j i d q1 q2", i=BLOC, d=2, q1=4, q2=4
        )
        for d in range(2):
            in1 = l1v[:, :, 2 * d, :].unsqueeze(3).broadcast_to([J, BLOC, 4, 4])
            in2 = l1v[:, :, 2 * d + 1, :].unsqueeze(2).broadcast_to([J, BLOC, 4, 4])
            o2 = o2all[:, :, d, :, :]
            nc.vector.tensor_tensor(out=o2, in0=in1, in1=in2, op=ALU.mult)

    # --- L3 chunked by batch; per-chunk U tiles so PE/ACT pipeline per chunk
    fin = wpool.tile([J, F_COLS], F32, tag="fin")  # 0:16 sumsq, 16:32 amp
    chunk_sizes = CHUNKS
    assert sum(chunk_sizes) == BLOC
    sq_dve = set(DVE_SQ_IDS)
    with tc.tile_pool(name="dsqp", bufs=2) as spool:
        i0 = 0
        for c, csz in enumerate(chunk_sizes):
            cw = csz * 256
            uc = [
                wpool.tile([J, cw], MM_DT, tag=f"u_{g}_{c}", name=f"u_{g}_{c}")
                for g in range(2)
            ]
            for g in range(2):
                l2v = l2[g][:].rearrange(
                    "j (i d c16) -> j i d c16", i=BLOC, d=2, c16=16
                )
                in1 = (
                    l2v[:, i0 : i0 + csz, 0, :]
                    .unsqueeze(3)
                    .broadcast_to([J, csz, 16, 16])
                )
                in2 = (
                    l2v[:, i0 : i0 + csz, 1, :]
                    .unsqueeze(2)
                    .broadcast_to([J, csz, 16, 16])
                )
                ov = uc[g][:].rearrange(
                    "j (i u1 u2) -> j i u1 u2", i=csz, u1=16, u2=16
                )
                eng = nc.gpsimd if f"{c}.{g}" in POOL_U else nc.vector
                eng.tensor_tensor(out=ov, in0=in1, in1=in2, op=ALU.mult)

            # amp partials for this chunk: fin[:, 16+i] = cU1[j,i,0]*U2[j,i,0]
            u1v = uc[0][:].rearrange("j (i t) -> j i t", i=csz, t=256)
            u2v = uc[1][:].rearrange("j (i t) -> j i t", i=csz, t=256)
            nc.vector.tensor_tensor(
                out=fin[:, 16 + i0 : 16 + i0 + csz],
                in0=u1v[:, :, 0], in1=u2v[:, :, 0], op=ALU.mult,
            )

            # D matmuls + square/accum for this chunk's batch elements
            for k in range(csz):
                i = i0 + k
                dt = dpool.tile([J, 512], F32, tag="D")
                rhs = uc[1][:, k * 256 : (k + 1) * 256]
                for h in range(2):
                    lhsT = uc[0][:, k * 256 + h * 128 : k * 256 + (h + 1) * 128]
                    nc.tensor.matmul(dt[:, h * 256 : (h + 1) * 256], lhsT, rhs)
                # norm partials: fin[:, i] = sum_T2 D[T1row, :]^2
                if i in sq_dve:
                    eng = nc.vector
                    # stage through SBUF in bf16 (PSUM dual-read is illegal;
                    # packed bf16 SBUF can hit the DVE 2x path)
                    dsq = spool.tile([J, 512], BF16, tag="dsq", name="dsq")
                    eng.tensor_copy(dsq[:], dt[:])
                    eng.scalar_tensor_tensor(
                        out=dsq[:], in0=dsq[:], scalar=1.0, in1=dsq[:],
                        op0=ALU.mult, op1=ALU.mult,
                        accum_out=fin[:, i : i + 1],
                    )
                else:
                    nc.scalar.activation(
                        dt[:], dt[:], AF.Square,
                        accum_out=fin[:, i : i + 1],
                    )
            i0 += csz

    # --- tail: ones-matmul reduces fin over partitions -> [1, 32]
    fout = fpool.tile([1, F_COLS], F32, tag="fout")
    nc.tensor.matmul(fout[:], params[:, PC_MASK : PC_MASK + 1], fin[:])
    nc.vector.tensor_copy(final[0:1, 0:32], fout[0:1, 0:32])

    # --- reg tail (off critical path; v-chain on Pool; Pool supports only
    # TensorTensor/TensorCopy, so the free-dim reduction goes through a
    # ones-matmul to [1,17] and the host sums those 17 values)
    sv = wpool.tile([37, 34], F32, tag="sv")
    nc.vector.tensor_copy(sv[:], fout_r[0:37, 0:34])
    v1 = wpool.tile([37, 17], F32, tag="v1")
    nc.gpsimd.tensor_tensor(out=v1[:], in0=sv[:, 0:17], in1=sv[:, 0:17], op=ALU.mult)
    v2 = wpool.tile([37, 17], F32, tag="v2")
    dvec_bc = params[0:37, PC_DVEC : PC_DVEC + 1].broadcast_to([37, 17])
    nc.gpsimd.tensor_tensor(out=v2[:], in0=v1[:], in1=dvec_bc, op=ALU.mult)
    v3 = wpool.tile([37, 17], F32, tag="v3")
    nc.gpsimd.tensor_tensor(out=v3[:], in0=v2[:], in1=sv[:, 17:34], op=ALU.subtract)
    v4 = wpool.tile([37, 17], F32, tag="v4")
    nc.gpsimd.tensor_tensor(
        out=v4[:], in0=v3[:],
        in1=params[0:37, PC_REGW : PC_REGW + 17], op=ALU.mult,
    )
    rt = fpool.tile([1, 17], F32, tag="rt")
    nc.tensor.matmul(rt[:], params[0:37, PC_MASK : PC_MASK + 1], v4[:])
    nc.vector.tensor_copy(final[0:1, 32:49], rt[:])

    nc.sync.dma_start(out_d[:, :], final[:])


def make_in_maps(input_ds, theta, coef):
    input_ds = np.asarray(input_ds, dtype=np.float32)
    theta = np.asarray(theta, dtype=np.float32)
    coef = np.asarray(coef, dtype=np.float32)
    pr = build_params()
    th = theta.transpose(1, 2, 0).reshape(J, L)
    pr[:, PC_THETA : PC_THETA + L] = th
    pr[:, PC_PHT : PC_PHT + L] = (np.float32(np.pi / 2.0) - th).astype(np.float32)
    pr[:, PC_COEF] = coef.reshape(J)
    in_maps = []
    for c in range(NCORES):
        blob = pr.copy()
        sl = input_ds[c * BLOC : (c + 1) * BLOC, :].reshape(1, BLOC * L)
        blob[:, PC_INDS:] = sl
        in_maps.append({"blob": blob})
    return in_maps


_NC_CACHE = None


def _get_program():
    global _NC_CACHE
    if _NC_CACHE is None:
        _NC_CACHE = build_program()
    return _NC_CACHE


def combine_outputs(results):
    """Host-side tail in float64: ln / mean over batch + reg term."""
    loss = 0.0
    for c in range(NCORES):
        o = np.asarray(results[c]["out"], dtype=np.float64)
        norm = o[0, 0:16]
        amp = o[0, 16:32]
        prob = amp * amp
        loss += float(np.sum(-(np.log(prob + EPS * norm) - np.log(norm))))
    loss /= float(B)
    loss += float(np.asarray(results[0]["out"], dtype=np.float64)[0, 32:49].sum())
    return np.float32(loss)


def kernel(input_ds, theta, coef):
    from concourse.bass_utils import run_bass_kernel_spmd

    nc = _get_program()
    in_maps = make_in_maps(input_ds, theta, coef)
    res = run_bass_kernel_spmd(nc, in_maps, core_ids=list(range(NCORES)))
    return combine_outputs(res.results)
